# revision 8
# baseline (speedup 1.0000x reference)
"""BitNet MLP (ternary gate/up GEMM + silu*up + Hadamard + act-quant + down GEMM)
on 8 Trainium2 NeuronCores.

Strategy: token-data-parallel across the 8 cores; each core runs T=1024 tokens
end-to-end so there is no large cross-core traffic (only a 4-float AllReduce
of |w| partial sums for the global ternarization scales).

Per core:
  - act_quant: per-token absmax -> int values in [-128,127] stored as bf16
    (rint computed exactly with the f32 +1.5*2^23 magic-add; matches
    jnp.round's half-to-even),
  - weights ternarized on the fly while streaming from HBM ({-1,0,1} bf16);
    all GEMMs are bf16 x bf16 with f32 PSUM accumulation -> bit-exact integer
    arithmetic at full tensor-engine rate,
  - fwht(8192) = H64 over the 64 I-chunks (f32 butterflies on vector/gpsimd)
    then H128 over partitions (one +/-1 matmul on the tensor engine),
  - all per-token dequant scales commute through act_quant (scale-invariant)
    and fold into a single per-token factor applied to the final output.
"""

import sys

sys.path.insert(0, "/opt/trn_rl_repo")

import numpy as np

import concourse.bass as bass
import concourse.mybir as mybir
import concourse.tile as tile
from concourse import bacc, bass_isa
from concourse.masks import make_identity

F32 = mybir.dt.float32
BF16 = mybir.dt.bfloat16
AX = mybir.AxisListType.X
OP = mybir.AluOpType
ACT_FN = mybir.ActivationFunctionType

MAGIC = 12582912.0  # 1.5*2^23: (x + MAGIC) - MAGIC == rint(x) in f32, |x|<2^22
EPS = 1e-5
QCLIP = 127.4375  # rint(min(t, QCLIP)) == min(rint(t), 127) exactly


def hadamard128():
    h = np.array([[1.0]], dtype=np.float32)
    while h.shape[0] < 128:
        h = np.block([[h, h], [h, -h]]).astype(np.float32)
    return h


def build_program(T, H, I, n_cores, sub=16):
    P = 128
    C = I // P           # I-chunks = H64 factor size
    HC = H // P          # H-chunks
    NT = min(512, T)     # token tile for matmul free dim
    TTN = T // NT
    n_tb = T // P
    n_sub = T // sub
    L = int(np.log2(C))
    assert 2 ** L == C and T % P == 0 and H % P == 0 and I % P == 0
    assert T % sub == 0 and H % 512 == 0 and I % 512 == 0
    inv_sqrt_i = float(1.0 / np.sqrt(I))
    wcount = float(I) * float(H)
    cg_n = max(1, 512 // sub)           # c-chunks per H128 matmul (N<=512)
    nxq = max(1, (HC + 1) // 2)         # h-chunks per xqT half

    nc = bacc.Bacc("TRN2", target_bir_lowering=False, num_devices=n_cores)

    x_d = nc.dram_tensor("x_s", [T, H], F32, kind="ExternalInput")
    wg_d = nc.dram_tensor("wg", [I, H], F32, kind="ExternalInput")
    wu_d = nc.dram_tensor("wu", [I, H], F32, kind="ExternalInput")
    wd_d = nc.dram_tensor("wd", [H, I], F32, kind="ExternalInput")
    wgs_d = nc.dram_tensor("wg_s", [I // n_cores, H], F32, kind="ExternalInput")
    wus_d = nc.dram_tensor("wu_s", [I // n_cores, H], F32, kind="ExternalInput")
    wds_d = nc.dram_tensor("wd_s", [H // n_cores, I], F32, kind="ExternalInput")
    hm_d = nc.dram_tensor("hmat", [P, P], F32, kind="ExternalInput")
    out_d = nc.dram_tensor("out_s", [T, H], F32, kind="ExternalOutput")

    cc_in = nc.dram_tensor("cc_in", [1, 4], F32)
    cc_out = nc.dram_tensor("cc_out", [1, 4], F32, addr_space="Shared")

    with tile.TileContext(nc) as tc:
        with (
            tc.tile_pool(name="consts", bufs=1) as consts,
            tc.tile_pool(name="wpipe", bufs=3) as wpipe,      # w f32 512-chunks
            tc.tile_pool(name="wqpipe", bufs=2) as wqpipe,    # wq bf16 512-chunks
            tc.tile_pool(name="wtpipe", bufs=2) as wtpipe,    # transposed lhsT
            tc.tile_pool(name="big16", bufs=2) as big16,      # xqT halves / wqdT
            tc.tile_pool(name="fw", bufs=1) as fw,            # fwht ping-pong
            tc.tile_pool(name="ip", bufs=1) as ip,            # interm / y_int
            tc.tile_pool(name="sc", bufs=1) as sc,            # scale rows
            tc.tile_pool(name="ep", bufs=2) as ep,            # GEMM epilogues
            tc.tile_pool(name="ps_mm", bufs=4, space="PSUM") as ps_mm,
            tc.tile_pool(name="ps_f", bufs=2, space="PSUM") as ps_f,
            tc.tile_pool(name="ps_tp", bufs=2, space="PSUM") as ps_tp,
        ):
            # ---------------- constants
            hmat = consts.tile([P, P], F32, tag="hmat")
            nc.sync.dma_start(hmat[:], hm_d.ap())
            ident_f = consts.tile([P, P], F32, tag="ident_f")
            make_identity(nc, ident_f[:])
            ident_b = consts.tile([P, P], BF16, tag="ident_b")
            nc.vector.tensor_copy(ident_b[:], ident_f[:])
            magicB = consts.tile([P, 1], F32, tag="magicB")
            nc.vector.memset(magicB[:], MAGIC)
            nmagicB = consts.tile([P, 1], F32, tag="nmagicB")
            nc.vector.memset(nmagicB[:], -MAGIC)

            # ---------------- weight-scale pass (shard |w| sums + AllReduce)
            def shard_abs_sum(src_d, rows, cols, tag):
                ntr, ntc = rows // P, cols // 512
                acc = sc.tile([P, ntr * ntc], F32, tag=f"acc_{tag}")
                for r in range(ntr):
                    for q in range(ntc):
                        t = wpipe.tile([P, 512], F32, tag="wf32")
                        nc.sync.dma_start(
                            t[:],
                            src_d.ap()[r * P:(r + 1) * P, q * 512:(q + 1) * 512])
                        nc.vector.tensor_reduce(
                            out=acc[:, r * ntc + q:r * ntc + q + 1], in_=t[:],
                            op=OP.add, axis=AX, apply_absolute_value=True)
                tot = sc.tile([P, 1], F32, tag=f"tot_{tag}")
                nc.vector.tensor_reduce(out=tot[:], in_=acc[:], op=OP.add, axis=AX)
                red = sc.tile([P, 1], F32, tag=f"red_{tag}")
                nc.gpsimd.partition_all_reduce(
                    red[:], tot[:], channels=P, reduce_op=bass_isa.ReduceOp.add)
                return red

            red_g = shard_abs_sum(wgs_d, I // n_cores, H, "g")
            red_u = shard_abs_sum(wus_d, I // n_cores, H, "u")
            red_d = shard_abs_sum(wds_d, H // n_cores, I, "d")

            ccin_sb = sc.tile([1, 4], F32, tag="ccin")
            nc.vector.memset(ccin_sb[:], 0.0)
            nc.vector.tensor_copy(ccin_sb[:, 0:1], red_g[0:1, :])
            nc.vector.tensor_copy(ccin_sb[:, 1:2], red_u[0:1, :])
            nc.vector.tensor_copy(ccin_sb[:, 2:3], red_d[0:1, :])
            nc.sync.dma_start(cc_in.ap(), ccin_sb[:])
            nc.gpsimd.collective_compute(
                "AllReduce", OP.add, ins=[cc_in.ap()], outs=[cc_out.ap()],
                replica_groups=[list(range(n_cores))])
            sums_sb = sc.tile([1, 4], F32, tag="sums")
            nc.sync.dma_start(sums_sb[:], cc_out.ap())

            wm_row = sc.tile([1, 4], F32, tag="wm_row")   # clip(mean|w|, eps)
            nc.vector.tensor_scalar(wm_row[:], sums_sb[:], 1.0 / wcount, EPS,
                                    OP.mult, OP.max)
            ws_row = sc.tile([1, 4], F32, tag="ws_row")   # 1/clip(mean|w|,eps)
            nc.vector.reciprocal(ws_row[:], wm_row[:])
            wsB = sc.tile([P, 4], F32, tag="wsB")
            nc.gpsimd.partition_broadcast(wsB[:], ws_row[:])

            # ---------------- x: act-quant + transpose into xqT
            xqt = [big16.tile([P, nxq, T], BF16, tag="big16", name=f"xqt{_i}")
                   for _i in range((HC + nxq - 1) // nxq)]
            am_row = sc.tile([1, T], F32, tag="am_row")
            for tb in range(n_tb):
                am4 = sc.tile([P, H // 512], F32, tag="am4")
                for q in range(H // 512):
                    xt = wpipe.tile([P, 512], F32, tag="wf32", name=f"xa{tb}_{q}")
                    nc.sync.dma_start(
                        xt[:], x_d.ap()[tb * P:(tb + 1) * P,
                                        q * 512:(q + 1) * 512])
                    nc.vector.tensor_reduce(
                        out=am4[:, q:q + 1], in_=xt[:], op=OP.max, axis=AX,
                        apply_absolute_value=True)
                amc = sc.tile([P, 1], F32, tag="amc")
                nc.vector.tensor_reduce(out=amc[:], in_=am4[:], op=OP.max,
                                        axis=AX)
                nc.vector.tensor_scalar(amc[:], amc[:], EPS, None, OP.max)
                sx = sc.tile([P, 1], F32, tag="sx")
                nc.vector.reciprocal(sx[:], amc[:])
                nc.vector.tensor_scalar(sx[:], sx[:], 128.0, None, OP.mult)
                for q in range(H // 512):
                    xt = wpipe.tile([P, 512], F32, tag="wf32", name=f"xb{tb}_{q}")
                    nc.sync.dma_start(
                        xt[:], x_d.ap()[tb * P:(tb + 1) * P,
                                        q * 512:(q + 1) * 512])
                    nc.scalar.activation(xt[:], xt[:], ACT_FN.Identity,
                                         bias=magicB[:], scale=sx[:])
                    xqb = wqpipe.tile([P, 512], BF16, tag="wq", name=f"xq{tb}_{q}")
                    nc.vector.tensor_scalar(xqb[:], xt[:], -MAGIC, 127.0,
                                            OP.add, OP.min)
                    for k in range(4):
                        hc = q * 4 + k
                        pt = ps_tp.tile([P, P], BF16, tag="tpb")
                        nc.tensor.transpose(pt[:], xqb[:, k * P:(k + 1) * P],
                                            ident_b[:])
                        nc.scalar.copy(
                            xqt[hc // nxq][:, hc % nxq, tb * P:(tb + 1) * P],
                            pt[:])
                pr = ps_tp.tile([P, P], F32, tag="tpb")
                nc.tensor.transpose(pr[:1, :], amc[:], ident_f[:])
                nc.scalar.copy(am_row[:, tb * P:(tb + 1) * P], pr[:1, :])

            # cgB = bcast(am_row * wm_g/128); amf_row doubles as scratch
            # (its real writes, in the FWHT phase, come after this broadcast)
            amf_row = sc.tile([1, T], F32, tag="amf_row")
            sg = sc.tile([1, 1], F32, tag="sg")
            nc.vector.tensor_scalar(sg[:], wm_row[:, 0:1], 1.0 / 128.0, None,
                                    OP.mult)
            nc.vector.tensor_scalar(amf_row[:], am_row[:], sg[:], None, OP.mult)
            bcastT = sc.tile([P, T], F32, tag="bcastT")
            nc.gpsimd.partition_broadcast(bcastT[:], amf_row[:])

            # ---------------- ternarize stream helper
            def tern_chunks(src_d, row0, cols, ws_ap):
                for q in range(cols // 512):
                    t = wpipe.tile([P, 512], F32, tag="wf32")
                    nc.sync.dma_start(
                        t[:], src_d.ap()[row0:row0 + P, q * 512:(q + 1) * 512])
                    nc.vector.tensor_scalar(t[:], t[:], ws_ap, -1.4375,
                                            OP.mult, OP.max)
                    nc.vector.tensor_scalar(t[:], t[:], 1.4375, MAGIC,
                                            OP.min, OP.add)
                    wq = wqpipe.tile([P, 512], BF16, tag="wq")
                    nc.scalar.activation(wq[:], t[:], ACT_FN.Identity,
                                         bias=nmagicB[:])
                    yield q, wq

            # ---------------- GEMM1 -> interm bf16 [P, C, T]
            interm = ip.tile([P, C, T], BF16, tag="interm")
            for ic in range(C):
                psg = [ps_mm.tile([P, NT], F32, tag="psmm", name=f"psg{ic}_{_i}")
                       for _i in range(TTN)]
                psu = [ps_mm.tile([P, NT], F32, tag="psmm", name=f"psu{ic}_{_i}")
                       for _i in range(TTN)]
                for mat_i, (w_d, ws_k, ps) in enumerate(
                        [(wg_d, 0, psg), (wu_d, 1, psu)]):
                    wqt = wtpipe.tile([P, HC, P], BF16, tag="wqt")
                    for q, wq in tern_chunks(w_d, ic * P, H, wsB[:, ws_k:ws_k + 1]):
                        for k in range(4):
                            nc.sync.dma_start(wqt[:, q * 4 + k, :],
                                              wq[:, k * P:(k + 1) * P],
                                              transpose=True)
                    for tt in range(TTN):
                        for hc in range(HC):
                            nc.tensor.matmul(
                                ps[tt][:], wqt[:, hc, :],
                                xqt[hc // nxq][:, hc % nxq,
                                               tt * NT:(tt + 1) * NT],
                                start=(hc == 0), stop=(hc == HC - 1))
                for tt in range(TTN):
                    g1 = ep.tile([P, NT], F32, tag="g1")
                    nc.vector.tensor_tensor(g1[:], psg[tt][:],
                                            bcastT[:, tt * NT:(tt + 1) * NT],
                                            OP.mult)
                    sgm = ep.tile([P, NT], F32, tag="sgm")
                    nc.scalar.activation(sgm[:], g1[:], ACT_FN.Sigmoid)
                    nc.vector.tensor_tensor(g1[:], g1[:], psu[tt][:], OP.mult)
                    nc.vector.tensor_tensor(
                        interm[:, ic, tt * NT:(tt + 1) * NT], g1[:], sgm[:],
                        OP.mult)

            # ---------------- FWHT + act-quant (in-place into interm)
            for s_i in range(n_sub):
                cols = slice(s_i * sub, (s_i + 1) * sub)
                b0 = fw.tile([P, C, sub], F32, tag="fw0")
                b1 = fw.tile([P, C, sub], F32, tag="fw1")
                bufs = [b0, b1]
                src = interm[:, :, cols]
                if L == 0:
                    nc.vector.tensor_copy(b0[:], src)
                    src = b0[:]
                for st in range(L):
                    h = 1 << st
                    dst = bufs[st % 2]
                    eng = nc.gpsimd if (L >= 5 and st in (1, 3)) else nc.vector
                    sview = src.rearrange("p (b two h) t -> p b two h t",
                                          two=2, h=h)
                    dview = dst[:].rearrange("p (b two h) t -> p b two h t",
                                             two=2, h=h)
                    eng.tensor_tensor(dview[:, :, 0], sview[:, :, 0],
                                      sview[:, :, 1], OP.add)
                    eng.tensor_tensor(dview[:, :, 1], sview[:, :, 0],
                                      sview[:, :, 1], OP.subtract)
                    src = dst[:]
                y2 = src
                fin = bufs[L % 2] if L > 0 else b1
                for cg0 in range(0, C, cg_n):
                    cgw = min(cg_n, C - cg0)
                    pf = ps_f.tile([P, 512], F32, tag="psf")
                    rhs = y2[:, cg0:cg0 + cgw, :].rearrange("p c t -> p (c t)")
                    nc.tensor.matmul(pf[:, :cgw * sub], hmat[:], rhs,
                                     start=True, stop=True)
                    nc.scalar.copy(
                        fin[:, cg0:cg0 + cgw, :].rearrange("p c t -> p (c t)"),
                        pf[:, :cgw * sub])
                m1r = sc.tile([P, sub], F32, tag="m1r")
                nc.vector.tensor_reduce(
                    out=m1r[:], in_=fin[:].rearrange("p c t -> p t c"),
                    op=OP.max, axis=AX, apply_absolute_value=True)
                am2 = sc.tile([P, sub], F32, tag="am2")
                nc.gpsimd.partition_all_reduce(
                    am2[:], m1r[:], channels=P,
                    reduce_op=bass_isa.ReduceOp.max)
                nc.vector.tensor_scalar(am2[:], am2[:], EPS, None, OP.max)
                nc.vector.tensor_copy(amf_row[:, cols], am2[0:1, :])
                rc2 = sc.tile([P, sub], F32, tag="rc2")
                nc.vector.reciprocal(rc2[:], am2[:])
                nc.vector.tensor_scalar(rc2[:], rc2[:], 128.0, None, OP.mult)
                other = b0 if L == 0 else bufs[(L + 1) % 2]
                nc.vector.tensor_tensor(
                    other[:], fin[:],
                    rc2[:, None, :].to_broadcast((P, C, sub)), OP.mult)
                nc.vector.tensor_scalar(other[:], other[:], QCLIP, MAGIC,
                                        OP.min, OP.add)
                nc.vector.tensor_scalar(interm[:, :, cols], other[:], -MAGIC,
                                        None, OP.add)

            # final per-token output scale
            sf = sc.tile([1, 1], F32, tag="sf")
            nc.vector.tensor_tensor(sf[:], wm_row[:, 2:3], wm_row[:, 1:2],
                                    OP.mult)
            nc.vector.tensor_scalar(sf[:], sf[:], inv_sqrt_i / (128.0 * 128.0),
                                    None, OP.mult)
            nc.vector.tensor_scalar(am_row[:], am_row[:], sf[:], None, OP.mult)
            nc.vector.tensor_tensor(am_row[:], am_row[:], amf_row[:], OP.mult)
            nc.gpsimd.partition_broadcast(bcastT[:], am_row[:])

            # ---------------- GEMM2 + transpose-out
            for hc in range(HC):
                wqdt = big16.tile([P, C, P], BF16, tag="big16")
                for q, wq in tern_chunks(wd_d, hc * P, I, wsB[:, 2:3]):
                    for k in range(4):
                        nc.sync.dma_start(wqdt[:, q * 4 + k, :],
                                          wq[:, k * P:(k + 1) * P],
                                          transpose=True)
                for tt in range(TTN):
                    pso = ps_mm.tile([P, NT], F32, tag="psmm")
                    for c in range(C):
                        nc.tensor.matmul(pso[:], wqdt[:, c, :],
                                         interm[:, c, tt * NT:(tt + 1) * NT],
                                         start=(c == 0), stop=(c == C - 1))
                    o1 = ep.tile([P, NT], F32, tag="g1")
                    nc.vector.tensor_tensor(o1[:], pso[:],
                                            bcastT[:, tt * NT:(tt + 1) * NT],
                                            OP.mult)
                    for k in range(NT // P):
                        tb = tt * (NT // P) + k
                        po = ps_tp.tile([P, P], F32, tag="tpb")
                        nc.tensor.transpose(po[:], o1[:, k * P:(k + 1) * P],
                                            ident_f[:])
                        ot = ep.tile([P, P], F32, tag="ot")
                        nc.scalar.copy(ot[:], po[:])
                        nc.sync.dma_start(
                            out_d.ap()[tb * P:(tb + 1) * P,
                                       hc * P:(hc + 1) * P], ot[:])

    nc.compile()
    return nc


_PROG_CACHE = {}
_LAST_IN_MAPS = None


def kernel(x, w_gate, w_up, w_down):
    from concourse.bass_utils import run_bass_kernel_spmd

    B, S, H = x.shape
    I = w_gate.shape[0]
    n_cores = 8
    M = B * S
    T = M // n_cores

    key = (T, H, I, n_cores)
    if key not in _PROG_CACHE:
        _PROG_CACHE[key] = build_program(T, H, I, n_cores)
    nc = _PROG_CACHE[key]

    xf = np.ascontiguousarray(x.reshape(M, H).astype(np.float32))
    hm = hadamard128()
    in_maps = []
    for c in range(n_cores):
        in_maps.append({
            "x_s": xf[c * T:(c + 1) * T],
            "wg": w_gate, "wu": w_up, "wd": w_down,
            "wg_s": np.ascontiguousarray(
                w_gate[c * (I // n_cores):(c + 1) * (I // n_cores)]),
            "wu_s": np.ascontiguousarray(
                w_up[c * (I // n_cores):(c + 1) * (I // n_cores)]),
            "wd_s": np.ascontiguousarray(
                w_down[c * (H // n_cores):(c + 1) * (H // n_cores)]),
            "hmat": hm,
        })
    global _LAST_IN_MAPS
    _LAST_IN_MAPS = in_maps
    res = run_bass_kernel_spmd(nc, in_maps, list(range(n_cores)))
    out = np.concatenate([res.results[c]["out_s"] for c in range(n_cores)], 0)
    return out.reshape(B, S, H).astype(np.float32)
